# revision 7
# baseline (speedup 1.0000x reference)
"""Trainium2 Bass kernel for a 3x3 stride-1 pad-1 conv:
x (32,128,64,64) f32, weight (256,128,3,3) f32, bias (256,) f32
-> out (32,256,64,64) f32.

Strategy: data-parallel over batch across 8 NeuronCores (4 samples each).
Per core, the conv is 9 shifted matmuls accumulating in PSUM:
  out[co, hw] = sum_{kh,kw} W[co, :, kh, kw] @ xpad[:, h+kh, w+kw]
C_in=128 sits on the SBUF partition dim; the moving operand is a
[128, 8*64] window of the zero-padded image (rows strided by 66), and the
stationary operand is the [ci, co] transpose of one (kh,kw) weight slice.
Matmuls run as float32r (full-rate fp32 mode; 4x faster than plain fp32).
"""

import numpy as np

import concourse.bass as bass
from concourse import bacc
import concourse.mybir as mybir
import concourse.tile as tile
from concourse.bass_utils import run_bass_kernel_spmd
from concourse.masks import make_identity

N_CORES = 8
B_FULL = 32
B_LOCAL = B_FULL // N_CORES  # 4
CI = 128
CO = 256
H = W = 64
HP = WP = 66  # zero-padded image
ROWS = 8  # output rows per PSUM tile -> free dim 8*64 = 512
N_T = H // ROWS
F32 = mybir.dt.float32
F32R = mybir.dt.float32r


def build_nc():
    nc = bacc.Bacc()
    x_d = nc.dram_tensor("x", [B_LOCAL, CI, H, W], F32, kind="ExternalInput")
    w_d = nc.dram_tensor("weight", [CO, CI, 3, 3], F32, kind="ExternalInput")
    b_d = nc.dram_tensor("bias", [CO], F32, kind="ExternalInput")
    o_d = nc.dram_tensor("out", [B_LOCAL, CO, H, W], F32, kind="ExternalOutput")

    with tile.TileContext(nc) as tc:
        with (
            tc.tile_pool(name="const", bufs=1) as const,
            tc.tile_pool(name="xstage", bufs=2) as xstage,
            tc.tile_pool(name="xpad", bufs=2) as xpool,
            tc.tile_pool(name="obuf", bufs=4) as opool,
            tc.tile_pool(name="psum", bufs=6, space="PSUM") as pspool,
            tc.tile_pool(name="psum_tr", bufs=2, space="PSUM") as trpool,
        ):
            # Load weight as [co_p, cb, ci*9] (contiguous per partition), then
            # transpose each 128x128 (kh,kw,cb) slice on the PE to [ci, co_p].
            w_raw = const.tile([128, 2, CI * 9], F32)
            nc.sync.dma_start(
                w_raw,
                w_d.rearrange("(cb cp) ci kh kw -> cp cb (ci kh kw)", cb=2),
            )
            ident = const.tile([128, 128], F32)
            make_identity(nc, ident)
            bias_sb = const.tile([128, 2], F32)
            nc.sync.dma_start(bias_sb, b_d.rearrange("(cb cp) -> cp cb", cb=2))

            # fp32r operands must come from a compute op that rounds to fp32r,
            # so matmul inputs are produced by DVE copies into float32r tiles.
            w_t = const.tile([128, 18, 128], F32R)  # [ci, cb*9+k, co_p]
            for cb in range(2):
                w_cb = w_raw[:, cb, :].rearrange("p (ci k) -> p k ci", k=9)
                for k in range(9):
                    ptr = trpool.tile([128, 128], F32)
                    nc.tensor.transpose(ptr, w_cb[:, k, :], ident)
                    nc.vector.tensor_copy(w_t[:, cb * 9 + k, :], ptr)

            o_v = o_d.rearrange("b (cb cp) h w -> b cb cp (h w)", cb=2)
            for b in range(B_LOCAL):
                xs = xstage.tile([128, HP, WP], F32)
                nc.vector.memset(xs[:, 0, :], 0.0)
                nc.vector.memset(xs[:, HP - 1, :], 0.0)
                nc.vector.memset(xs[:, :, 0], 0.0)
                nc.vector.memset(xs[:, :, WP - 1], 0.0)
                nc.sync.dma_start(xs[:, 1 : H + 1, 1 : W + 1], x_d[b])
                xp = xpool.tile([128, HP, WP], F32R)
                nc.vector.tensor_copy(xp, xs)

                for cb in range(2):
                    for t in range(N_T):
                        h0 = t * ROWS
                        ps = pspool.tile([128, ROWS * W], F32)
                        for k in range(9):
                            kh, kw = divmod(k, 3)
                            rhs = xp[:, h0 + kh : h0 + kh + ROWS, kw : kw + W]
                            nc.tensor.matmul(
                                ps,
                                w_t[:, cb * 9 + k, :],
                                rhs,
                                start=(k == 0),
                                stop=(k == 8),
                            )
                        ob = opool.tile([128, ROWS * W], F32)
                        nc.scalar.activation(
                            ob,
                            ps,
                            mybir.ActivationFunctionType.Identity,
                            bias=bias_sb[:, cb : cb + 1],
                        )
                        nc.sync.dma_start(
                            o_v[b, cb, :, h0 * W : (h0 + ROWS) * W], ob
                        )

    nc.finalize()
    return nc


def run(x: np.ndarray, weight: np.ndarray, bias: np.ndarray, **spmd_kwargs):
    x = np.ascontiguousarray(x, dtype=np.float32)
    weight = np.ascontiguousarray(weight, dtype=np.float32)
    bias = np.ascontiguousarray(bias, dtype=np.float32)

    nc = build_nc()
    in_maps = [
        {
            "x": x[c * B_LOCAL : (c + 1) * B_LOCAL],
            "weight": weight,
            "bias": bias,
        }
        for c in range(N_CORES)
    ]
    res = run_bass_kernel_spmd(
        nc, in_maps, core_ids=list(range(N_CORES)), **spmd_kwargs
    )
    out = np.concatenate([r["out"] for r in res.results], axis=0)
    return out, res


def kernel(x: np.ndarray, weight: np.ndarray, bias: np.ndarray) -> np.ndarray:
    out, _ = run(x, weight, bias)
    return out


# revision 9
# speedup vs baseline: 1.0063x; 1.0063x over previous
"""Trainium2 Bass kernel for a 3x3 stride-1 pad-1 conv:
x (32,128,64,64) f32, weight (256,128,3,3) f32, bias (256,) f32
-> out (32,256,64,64) f32.

Strategy: data-parallel over batch across 8 NeuronCores (4 samples each).
Per core, the conv is 9 shifted matmuls accumulating in PSUM:
  out[co, hw] = sum_{kh,kw} W[co, :, kh, kw] @ xpad[:, h+kh, w+kw]
C_in=128 sits on the SBUF partition dim; the moving operand is a
[128, 8*64] window of the zero-padded image (rows strided by 66), and the
stationary operand is the [ci, co] transpose of one (kh,kw) weight slice.
Matmuls run as float32r (full-rate fp32 mode; 4x faster than plain fp32).
"""

import numpy as np

import concourse.bass as bass
from concourse import bacc
import concourse.mybir as mybir
import concourse.tile as tile
from concourse.bass_utils import run_bass_kernel_spmd
from concourse.masks import make_identity

N_CORES = 8
B_FULL = 32
B_LOCAL = B_FULL // N_CORES  # 4
CI = 128
CO = 256
H = W = 64
HP = WP = 66  # zero-padded image
ROWS = 8  # output rows per PSUM tile -> free dim 8*64 = 512
N_T = H // ROWS
F32 = mybir.dt.float32
F32R = mybir.dt.float32r


def build_nc():
    nc = bacc.Bacc()
    x_d = nc.dram_tensor("x", [B_LOCAL, CI, H, W], F32, kind="ExternalInput")
    w_d = nc.dram_tensor("weight", [CO, CI, 3, 3], F32, kind="ExternalInput")
    b_d = nc.dram_tensor("bias", [CO], F32, kind="ExternalInput")
    o_d = nc.dram_tensor("out", [B_LOCAL, CO, H, W], F32, kind="ExternalOutput")

    with tile.TileContext(nc) as tc:
        with (
            tc.tile_pool(name="const", bufs=1) as const,
            tc.tile_pool(name="xstage", bufs=B_LOCAL) as xstage,
            tc.tile_pool(name="xpad", bufs=B_LOCAL) as xpool,
            tc.tile_pool(name="obuf", bufs=4) as opool,
            tc.tile_pool(name="psum", bufs=6, space="PSUM") as pspool,
            tc.tile_pool(name="psum_tr", bufs=2, space="PSUM") as trpool,
        ):
            # Load weight as [co_p, cb, ci*9] (contiguous per partition), then
            # transpose each 128x128 (kh,kw,cb) slice on the PE to [ci, co_p].
            # Input loads ride the SWDGE (gpsimd) ring so they never queue
            # behind the HWDGE (sync) ring carrying the 64 output stores.
            w_raw = const.tile([128, 2, CI * 9], F32)
            nc.gpsimd.dma_start(
                w_raw,
                w_d.rearrange("(cb cp) ci kh kw -> cp cb (ci kh kw)", cb=2),
            )
            ident = const.tile([128, 128], F32)
            make_identity(nc, ident)
            bias_sb = const.tile([128, 2], F32)
            nc.gpsimd.dma_start(bias_sb, b_d.rearrange("(cb cp) -> cp cb", cb=2))

            # Prefetch ALL samples up-front: contiguous DMA into an fp32 stage,
            # then a DVE copy does padding insertion + the required fp32r
            # rounding (fp32r matmul operands must be produced by a compute op).
            # Sample 0 is split in two chunks so compute starts sooner.
            x_v = x_d.rearrange("b c h w -> b c (h w)")
            # memset cannot target f32r tiles; borders get zeroed via a
            # tensor_copy from this fp32 zero row (a valid f32r producer).
            zrow = const.tile([128, WP], F32)
            nc.vector.memset(zrow, 0.0)
            xps = []
            for b in range(B_LOCAL):
                xin = xstage.tile([128, H * W], F32)
                xp = xpool.tile([128, HP, WP], F32R)
                nc.vector.tensor_copy(xp[:, 0, :], zrow)
                nc.vector.tensor_copy(xp[:, HP - 1, :], zrow)
                nc.vector.tensor_copy(xp[:, :, 0], zrow)
                nc.vector.tensor_copy(xp[:, :, WP - 1], zrow)
                n_chunks = 2 if b == 0 else 1
                rows_per = H // n_chunks
                for c in range(n_chunks):
                    r0 = c * rows_per
                    nc.gpsimd.dma_start(
                        xin[:, r0 * W : (r0 + rows_per) * W],
                        x_v[b, :, r0 * W : (r0 + rows_per) * W],
                    )
                    nc.vector.tensor_copy(
                        xp[:, 1 + r0 : 1 + r0 + rows_per, 1 : W + 1],
                        xin[:, r0 * W : (r0 + rows_per) * W].rearrange(
                            "p (h w) -> p h w", w=W
                        ),
                    )
                xps.append(xp)

            w_t = const.tile([128, 18, 128], F32R)  # [ci, cb*9+k, co_p]
            for cb in range(2):
                w_cb = w_raw[:, cb, :].rearrange("p (ci k) -> p k ci", k=9)
                for k in range(9):
                    ptr = trpool.tile([128, 128], F32)
                    nc.tensor.transpose(ptr, w_cb[:, k, :], ident)
                    nc.vector.tensor_copy(w_t[:, cb * 9 + k, :], ptr)

            o_v = o_d.rearrange("b (cb cp) h w -> b cb cp (h w)", cb=2)
            for b in range(B_LOCAL):
                xp = xps[b]
                for cb in range(2):
                    for t in range(N_T):
                        h0 = t * ROWS
                        ps = pspool.tile([128, ROWS * W], F32)
                        for k in range(9):
                            kh, kw = divmod(k, 3)
                            rhs = xp[:, h0 + kh : h0 + kh + ROWS, kw : kw + W]
                            nc.tensor.matmul(
                                ps,
                                w_t[:, cb * 9 + k, :],
                                rhs,
                                start=(k == 0),
                                stop=(k == 8),
                            )
                        ob = opool.tile([128, ROWS * W], F32)
                        nc.scalar.activation(
                            ob,
                            ps,
                            mybir.ActivationFunctionType.Identity,
                            bias=bias_sb[:, cb : cb + 1],
                        )
                        nc.sync.dma_start(
                            o_v[b, cb, :, h0 * W : (h0 + ROWS) * W], ob
                        )

    nc.finalize()
    return nc


def run(x: np.ndarray, weight: np.ndarray, bias: np.ndarray, **spmd_kwargs):
    x = np.ascontiguousarray(x, dtype=np.float32)
    weight = np.ascontiguousarray(weight, dtype=np.float32)
    bias = np.ascontiguousarray(bias, dtype=np.float32)

    nc = build_nc()
    in_maps = [
        {
            "x": x[c * B_LOCAL : (c + 1) * B_LOCAL],
            "weight": weight,
            "bias": bias,
        }
        for c in range(N_CORES)
    ]
    res = run_bass_kernel_spmd(
        nc, in_maps, core_ids=list(range(N_CORES)), **spmd_kwargs
    )
    out = np.concatenate([r["out"] for r in res.results], axis=0)
    return out, res


def kernel(x: np.ndarray, weight: np.ndarray, bias: np.ndarray) -> np.ndarray:
    out, _ = run(x, weight, bias)
    return out


# revision 12
# speedup vs baseline: 1.0276x; 1.0211x over previous
"""Trainium2 Bass kernel for a 3x3 stride-1 pad-1 conv:
x (32,128,64,64) f32, weight (256,128,3,3) f32, bias (256,) f32
-> out (32,256,64,64) f32.

Strategy: data-parallel over batch across 8 NeuronCores (4 samples each).
Per core, the conv is 9 shifted matmuls accumulating in PSUM:
  out[co, hw] = sum_{kh,kw} W[co, :, kh, kw] @ xpad[:, h+kh, w+kw]
C_in=128 sits on the SBUF partition dim; the moving operand is a
[128, 8*64] window of the zero-padded image (rows strided by 66), and the
stationary operand is the [ci, co] transpose of one (kh,kw) weight slice.
Matmuls run as float32r (full-rate fp32 mode; 4x faster than plain fp32).
"""

import numpy as np

import concourse.bass as bass
from concourse import bacc
import concourse.mybir as mybir
import concourse.tile as tile
from concourse.bass_utils import run_bass_kernel_spmd
from concourse.masks import make_identity

N_CORES = 8
B_FULL = 32
B_LOCAL = B_FULL // N_CORES  # 4
CI = 128
CO = 256
H = W = 64
HP = WP = 66  # zero-padded image
ROWS = 8  # output rows per PSUM tile -> free dim 8*64 = 512
N_T = H // ROWS
F32 = mybir.dt.float32
F32R = mybir.dt.float32r


def build_nc():
    nc = bacc.Bacc()
    x_d = nc.dram_tensor("x", [B_LOCAL, CI, H, W], F32, kind="ExternalInput")
    w_d = nc.dram_tensor("weight", [CO, CI, 3, 3], F32, kind="ExternalInput")
    b_d = nc.dram_tensor("bias", [CO], F32, kind="ExternalInput")
    o_d = nc.dram_tensor("out", [B_LOCAL, CO, H, W], F32, kind="ExternalOutput")

    with tile.TileContext(nc) as tc:
        with (
            tc.tile_pool(name="const", bufs=1) as const,
            tc.tile_pool(name="xstage", bufs=B_LOCAL) as xstage,
            tc.tile_pool(name="xpad", bufs=B_LOCAL) as xpool,
            tc.tile_pool(name="obuf", bufs=4) as opool,
            tc.tile_pool(name="psum", bufs=6, space="PSUM") as pspool,
            tc.tile_pool(name="psum_tr", bufs=2, space="PSUM") as trpool,
        ):
            # Load weight as [co_p, cb, ci*9] (contiguous per partition), then
            # transpose each 128x128 (kh,kw,cb) slice on the PE to [ci, co_p].
            # Input loads ride the ACT HWDGE ring (qActDynamicHW) so they
            # never queue behind the sync HWDGE ring carrying the 64 output
            # stores (HWDGE is FIFO per issuing engine).
            w_raw = const.tile([128, 2, CI * 9], F32)
            w_v = w_d.rearrange("(cb cp) ci kh kw -> cp cb (ci kh kw)", cb=2)
            for cb in range(2):
                nc.scalar.dma_start(w_raw[:, cb], w_v[:, cb])
            ident = const.tile([128, 128], F32)
            make_identity(nc, ident)
            bias_sb = const.tile([128, 2], F32)
            nc.scalar.dma_start(bias_sb, b_d.rearrange("(cb cp) -> cp cb", cb=2))

            # Prefetch ALL samples up-front: contiguous DMA into an fp32 stage,
            # then a DVE copy does padding insertion + the required fp32r
            # rounding (fp32r matmul operands must be produced by a compute op).
            # Sample 0 is split in two chunks so compute starts sooner.
            x_v = x_d.rearrange("b c h w -> b c (h w)")
            # memset cannot target f32r tiles; borders get zeroed via a
            # tensor_copy from this fp32 zero row (a valid f32r producer).
            zrow = const.tile([128, WP], F32)
            nc.vector.memset(zrow, 0.0)
            xps = []
            for b in range(B_LOCAL):
                xin = xstage.tile([128, H * W], F32)
                xp = xpool.tile([128, HP, WP], F32R)
                nc.vector.tensor_copy(xp[:, 0, :], zrow)
                nc.vector.tensor_copy(xp[:, HP - 1, :], zrow)
                nc.vector.tensor_copy(xp[:, :, 0], zrow)
                nc.vector.tensor_copy(xp[:, :, WP - 1], zrow)
                n_chunks = 2 if b == 0 else 1
                rows_per = H // n_chunks
                for c in range(n_chunks):
                    r0 = c * rows_per
                    nc.scalar.dma_start(
                        xin[:, r0 * W : (r0 + rows_per) * W],
                        x_v[b, :, r0 * W : (r0 + rows_per) * W],
                    )
                    nc.vector.tensor_copy(
                        xp[:, 1 + r0 : 1 + r0 + rows_per, 1 : W + 1],
                        xin[:, r0 * W : (r0 + rows_per) * W].rearrange(
                            "p (h w) -> p h w", w=W
                        ),
                    )
                xps.append(xp)

            w_t = const.tile([128, 18, 128], F32R)  # [ci, cb*9+k, co_p]
            for cb in range(2):
                w_cb = w_raw[:, cb, :].rearrange("p (ci k) -> p k ci", k=9)
                for k in range(9):
                    ptr = trpool.tile([128, 128], F32)
                    nc.tensor.transpose(ptr, w_cb[:, k, :], ident)
                    nc.vector.tensor_copy(w_t[:, cb * 9 + k, :], ptr)

            o_v = o_d.rearrange("b (cb cp) h w -> b cb cp (h w)", cb=2)
            for b in range(B_LOCAL):
                xp = xps[b]
                for cb in range(2):
                    for t in range(N_T):
                        h0 = t * ROWS
                        ps = pspool.tile([128, ROWS * W], F32)
                        for k in range(9):
                            kh, kw = divmod(k, 3)
                            rhs = xp[:, h0 + kh : h0 + kh + ROWS, kw : kw + W]
                            nc.tensor.matmul(
                                ps,
                                w_t[:, cb * 9 + k, :],
                                rhs,
                                start=(k == 0),
                                stop=(k == 8),
                            )
                        ob = opool.tile([128, ROWS * W], F32)
                        nc.scalar.activation(
                            ob,
                            ps,
                            mybir.ActivationFunctionType.Identity,
                            bias=bias_sb[:, cb : cb + 1],
                        )
                        nc.sync.dma_start(
                            o_v[b, cb, :, h0 * W : (h0 + ROWS) * W], ob
                        )

    nc.finalize()
    return nc


def run(x: np.ndarray, weight: np.ndarray, bias: np.ndarray, **spmd_kwargs):
    x = np.ascontiguousarray(x, dtype=np.float32)
    weight = np.ascontiguousarray(weight, dtype=np.float32)
    bias = np.ascontiguousarray(bias, dtype=np.float32)

    nc = build_nc()
    in_maps = [
        {
            "x": x[c * B_LOCAL : (c + 1) * B_LOCAL],
            "weight": weight,
            "bias": bias,
        }
        for c in range(N_CORES)
    ]
    res = run_bass_kernel_spmd(
        nc, in_maps, core_ids=list(range(N_CORES)), **spmd_kwargs
    )
    out = np.concatenate([r["out"] for r in res.results], axis=0)
    return out, res


def kernel(x: np.ndarray, weight: np.ndarray, bias: np.ndarray) -> np.ndarray:
    out, _ = run(x, weight, bias)
    return out


# revision 15
# speedup vs baseline: 1.0361x; 1.0083x over previous
"""Trainium2 Bass kernel for a 3x3 stride-1 pad-1 conv:
x (32,128,64,64) f32, weight (256,128,3,3) f32, bias (256,) f32
-> out (32,256,64,64) f32.

Strategy: data-parallel over batch across 8 NeuronCores (4 samples each).
Per core, the conv is 9 shifted matmuls accumulating in PSUM:
  out[co, hw] = sum_{kh,kw} W[co, :, kh, kw] @ xpad[:, h+kh, w+kw]
C_in=128 sits on the SBUF partition dim; the moving operand is a
[128, 8*64] window of the zero-padded image (rows strided by 66), and the
stationary operand is the [ci, co] transpose of one (kh,kw) weight slice.
Matmuls run as float32r (full-rate fp32 mode; 4x faster than plain fp32).
"""

import numpy as np

import concourse.bass as bass
from concourse import bacc
import concourse.mybir as mybir
import concourse.tile as tile
from concourse.bass_utils import run_bass_kernel_spmd
from concourse.masks import make_identity

N_CORES = 8
B_FULL = 32
B_LOCAL = B_FULL // N_CORES  # 4
CI = 128
CO = 256
H = W = 64
HP = WP = 66  # zero-padded image
ROWS = 8  # output rows per PSUM tile -> free dim 8*64 = 512
N_T = H // ROWS
F32 = mybir.dt.float32
F32R = mybir.dt.float32r


def build_nc():
    nc = bacc.Bacc()
    x_d = nc.dram_tensor("x", [B_LOCAL, CI, H, W], F32, kind="ExternalInput")
    w_d = nc.dram_tensor("weight", [CO, CI, 3, 3], F32, kind="ExternalInput")
    b_d = nc.dram_tensor("bias", [CO], F32, kind="ExternalInput")
    o_d = nc.dram_tensor("out", [B_LOCAL, CO, H, W], F32, kind="ExternalOutput")

    with tile.TileContext(nc) as tc:
        with (
            tc.tile_pool(name="const", bufs=1) as const,
            tc.tile_pool(name="xstage", bufs=B_LOCAL) as xstage,
            tc.tile_pool(name="xpad", bufs=B_LOCAL) as xpool,
            tc.tile_pool(name="obuf", bufs=4) as opool,
            tc.tile_pool(name="psum", bufs=6, space="PSUM") as pspool,
            tc.tile_pool(name="psum_tr", bufs=2, space="PSUM") as trpool,
        ):
            # Load weight as [co_p, cb, ci*9] (contiguous per partition), then
            # transpose each 128x128 (kh,kw,cb) slice on the PE to [ci, co_p].
            # Input loads ride the ACT HWDGE ring (qActDynamicHW) so they
            # never queue behind the sync HWDGE ring carrying the 64 output
            # stores (HWDGE is FIFO per issuing engine).
            ident = const.tile([128, 128], F32)
            make_identity(nc, ident)
            # PE_HAM flips the clock gate 1.2->2.4 GHz only after ~3.4us of
            # sustained PE activity; burn the unavoidable initial DMA wait on
            # dummy transposes so the real matmuls start at full clock.
            for _ in range(48):
                warm = trpool.tile([128, 128], F32, tag="tr")
                nc.tensor.transpose(warm, ident, ident)

            w_raw = const.tile([128, 2, CI * 9], F32)
            w_v = w_d.rearrange("(cb cp) ci kh kw -> cp cb (ci kh kw)", cb=2)
            nc.scalar.dma_start(w_raw[:, 0], w_v[:, 0])
            bias_sb = const.tile([128, 2], F32)

            # Prefetch ALL samples up-front: contiguous DMA into an fp32 stage,
            # then a DVE copy does padding insertion + the required fp32r
            # rounding (fp32r matmul operands must be produced by a compute op).
            # Sample 0 is split in two chunks so compute starts sooner.
            x_v = x_d.rearrange("b c h w -> b c (h w)")
            # memset cannot target f32r tiles; borders get zeroed via a
            # tensor_copy from this fp32 zero row (a valid f32r producer).
            zrow = const.tile([128, WP], F32)
            nc.vector.memset(zrow, 0.0)

            def load_sample(b, n_chunks):
                xin = xstage.tile([128, H * W], F32)
                xp = xpool.tile([128, HP, WP], F32R)
                nc.vector.tensor_copy(xp[:, 0, :], zrow)
                nc.vector.tensor_copy(xp[:, HP - 1, :], zrow)
                nc.vector.tensor_copy(xp[:, :, 0], zrow)
                nc.vector.tensor_copy(xp[:, :, WP - 1], zrow)
                rows_per = H // n_chunks
                for c in range(n_chunks):
                    r0 = c * rows_per
                    nc.scalar.dma_start(
                        xin[:, r0 * W : (r0 + rows_per) * W],
                        x_v[b, :, r0 * W : (r0 + rows_per) * W],
                    )
                    nc.vector.tensor_copy(
                        xp[:, 1 + r0 : 1 + r0 + rows_per, 1 : W + 1],
                        xin[:, r0 * W : (r0 + rows_per) * W].rearrange(
                            "p (h w) -> p h w", w=W
                        ),
                    )
                return xp

            # ACT-ring FIFO order: w half 0, sample-0 chunks, w half 1,
            # bias, then the remaining samples.
            xps = [load_sample(0, 2)]
            nc.scalar.dma_start(w_raw[:, 1], w_v[:, 1])
            nc.scalar.dma_start(bias_sb, b_d.rearrange("(cb cp) -> cp cb", cb=2))
            for b in range(1, B_LOCAL):
                xps.append(load_sample(b, 1))

            w_t = const.tile([128, 18, 128], F32R)  # [ci, cb*9+k, co_p]
            for cb in range(2):
                w_cb = w_raw[:, cb, :].rearrange("p (ci k) -> p k ci", k=9)
                for k in range(9):
                    ptr = trpool.tile([128, 128], F32, tag="tr")
                    nc.tensor.transpose(ptr, w_cb[:, k, :], ident)
                    nc.vector.tensor_copy(w_t[:, cb * 9 + k, :], ptr)

            o_v = o_d.rearrange("b (cb cp) h w -> b cb cp (h w)", cb=2)
            for b in range(B_LOCAL):
                xp = xps[b]
                for cb in range(2):
                    for t in range(N_T):
                        h0 = t * ROWS
                        ps = pspool.tile([128, ROWS * W], F32)
                        for k in range(9):
                            kh, kw = divmod(k, 3)
                            rhs = xp[:, h0 + kh : h0 + kh + ROWS, kw : kw + W]
                            nc.tensor.matmul(
                                ps,
                                w_t[:, cb * 9 + k, :],
                                rhs,
                                start=(k == 0),
                                stop=(k == 8),
                            )
                        ob = opool.tile([128, ROWS * W], F32)
                        nc.scalar.activation(
                            ob,
                            ps,
                            mybir.ActivationFunctionType.Identity,
                            bias=bias_sb[:, cb : cb + 1],
                        )
                        nc.sync.dma_start(
                            o_v[b, cb, :, h0 * W : (h0 + ROWS) * W], ob
                        )

    nc.finalize()
    return nc


def run(x: np.ndarray, weight: np.ndarray, bias: np.ndarray, **spmd_kwargs):
    x = np.ascontiguousarray(x, dtype=np.float32)
    weight = np.ascontiguousarray(weight, dtype=np.float32)
    bias = np.ascontiguousarray(bias, dtype=np.float32)

    nc = build_nc()
    in_maps = [
        {
            "x": x[c * B_LOCAL : (c + 1) * B_LOCAL],
            "weight": weight,
            "bias": bias,
        }
        for c in range(N_CORES)
    ]
    res = run_bass_kernel_spmd(
        nc, in_maps, core_ids=list(range(N_CORES)), **spmd_kwargs
    )
    out = np.concatenate([r["out"] for r in res.results], axis=0)
    return out, res


def kernel(x: np.ndarray, weight: np.ndarray, bias: np.ndarray) -> np.ndarray:
    out, _ = run(x, weight, bias)
    return out


# revision 16
# speedup vs baseline: 1.0448x; 1.0083x over previous
"""Trainium2 Bass kernel for a 3x3 stride-1 pad-1 conv:
x (32,128,64,64) f32, weight (256,128,3,3) f32, bias (256,) f32
-> out (32,256,64,64) f32.

Strategy: data-parallel over batch across 8 NeuronCores (4 samples each).
Per core, the conv is 9 shifted matmuls accumulating in PSUM:
  out[co, hw] = sum_{kh,kw} W[co, :, kh, kw] @ xpad[:, h+kh, w+kw]
C_in=128 sits on the SBUF partition dim; the moving operand is a
[128, 8*64] window of the zero-padded image (rows strided by 66), and the
stationary operand is the [ci, co] transpose of one (kh,kw) weight slice.
Matmuls run as float32r (full-rate fp32 mode; 4x faster than plain fp32).
"""

import numpy as np

import concourse.bass as bass
from concourse import bacc
import concourse.mybir as mybir
import concourse.tile as tile
from concourse.bass_utils import run_bass_kernel_spmd
from concourse.masks import make_identity

N_CORES = 8
B_FULL = 32
B_LOCAL = B_FULL // N_CORES  # 4
CI = 128
CO = 256
H = W = 64
HP = WP = 66  # zero-padded image
ROWS = 8  # output rows per PSUM tile -> free dim 8*64 = 512
N_T = H // ROWS
F32 = mybir.dt.float32
F32R = mybir.dt.float32r


def build_nc():
    nc = bacc.Bacc()
    x_d = nc.dram_tensor("x", [B_LOCAL, CI, H, W], F32, kind="ExternalInput")
    w_d = nc.dram_tensor("weight", [CO, CI, 3, 3], F32, kind="ExternalInput")
    b_d = nc.dram_tensor("bias", [CO], F32, kind="ExternalInput")
    o_d = nc.dram_tensor("out", [B_LOCAL, CO, H, W], F32, kind="ExternalOutput")

    with tile.TileContext(nc) as tc:
        with (
            tc.tile_pool(name="const", bufs=1) as const,
            tc.tile_pool(name="xstage", bufs=B_LOCAL) as xstage,
            tc.tile_pool(name="xpad", bufs=B_LOCAL) as xpool,
            tc.tile_pool(name="obuf", bufs=4) as opool,
            tc.tile_pool(name="psum", bufs=6, space="PSUM") as pspool,
            tc.tile_pool(name="psum_tr", bufs=2, space="PSUM") as trpool,
        ):
            # Load weight as [co_p, cb, ci*9] (contiguous per partition), then
            # transpose each 128x128 (kh,kw,cb) slice on the PE to [ci, co_p].
            # Input loads ride the ACT HWDGE ring (qActDynamicHW) so they
            # never queue behind the sync HWDGE ring carrying the 64 output
            # stores (HWDGE is FIFO per issuing engine).
            ident = const.tile([128, 128], F32)
            make_identity(nc, ident)
            # PE_HAM flips the clock gate 1.2->2.4 GHz only after ~3.4us of
            # sustained PE activity; burn the unavoidable initial DMA wait on
            # dummy transposes so the real matmuls start at full clock.
            for _ in range(8):
                warm = trpool.tile([128, 128], F32, tag="tr")
                nc.tensor.transpose(warm, ident, ident)

            w_raw = const.tile([128, 2, CI * 9], F32)
            w_v = w_d.rearrange("(cb cp) ci kh kw -> cp cb (ci kh kw)", cb=2)
            nc.scalar.dma_start(w_raw[:, 0], w_v[:, 0])
            bias_sb = const.tile([128, 2], F32)

            # Prefetch ALL samples up-front: contiguous DMA into an fp32 stage,
            # then a DVE copy does padding insertion + the required fp32r
            # rounding (fp32r matmul operands must be produced by a compute op).
            # Sample 0 is split in two chunks so compute starts sooner.
            x_v = x_d.rearrange("b c h w -> b c (h w)")
            # memset cannot target f32r tiles; borders get zeroed via a
            # tensor_copy from this fp32 zero row (a valid f32r producer).
            zrow = const.tile([128, WP], F32)
            nc.vector.memset(zrow, 0.0)

            def load_sample(b, n_chunks):
                xin = xstage.tile([128, H * W], F32)
                xp = xpool.tile([128, HP, WP], F32R)
                nc.vector.tensor_copy(xp[:, 0, :], zrow)
                nc.vector.tensor_copy(xp[:, HP - 1, :], zrow)
                nc.vector.tensor_copy(xp[:, :, 0], zrow)
                nc.vector.tensor_copy(xp[:, :, WP - 1], zrow)
                rows_per = H // n_chunks
                for c in range(n_chunks):
                    r0 = c * rows_per
                    nc.scalar.dma_start(
                        xin[:, r0 * W : (r0 + rows_per) * W],
                        x_v[b, :, r0 * W : (r0 + rows_per) * W],
                    )
                    nc.vector.tensor_copy(
                        xp[:, 1 + r0 : 1 + r0 + rows_per, 1 : W + 1],
                        xin[:, r0 * W : (r0 + rows_per) * W].rearrange(
                            "p (h w) -> p h w", w=W
                        ),
                    )
                return xp

            # ACT-ring FIFO order: w half 0, sample-0 chunks, w half 1,
            # bias, then the remaining samples.
            xps = [load_sample(0, 2)]
            nc.scalar.dma_start(w_raw[:, 1], w_v[:, 1])
            nc.scalar.dma_start(bias_sb, b_d.rearrange("(cb cp) -> cp cb", cb=2))
            for b in range(1, B_LOCAL):
                xps.append(load_sample(b, 1))

            w_t = const.tile([128, 18, 128], F32R)  # [ci, cb*9+k, co_p]
            for cb in range(2):
                w_cb = w_raw[:, cb, :].rearrange("p (ci k) -> p k ci", k=9)
                for k in range(9):
                    ptr = trpool.tile([128, 128], F32, tag="tr")
                    nc.tensor.transpose(ptr, w_cb[:, k, :], ident)
                    nc.vector.tensor_copy(w_t[:, cb * 9 + k, :], ptr)

            o_v = o_d.rearrange("b (cb cp) h w -> b cb cp (h w)", cb=2)
            for b in range(B_LOCAL):
                xp = xps[b]
                for cb in range(2):
                    for t in range(N_T):
                        h0 = t * ROWS
                        ps = pspool.tile([128, ROWS * W], F32)
                        for k in range(9):
                            kh, kw = divmod(k, 3)
                            rhs = xp[:, h0 + kh : h0 + kh + ROWS, kw : kw + W]
                            nc.tensor.matmul(
                                ps,
                                w_t[:, cb * 9 + k, :],
                                rhs,
                                start=(k == 0),
                                stop=(k == 8),
                            )
                        ob = opool.tile([128, ROWS * W], F32)
                        nc.vector.tensor_scalar_add(ob, ps, bias_sb[:, cb : cb + 1])
                        nc.sync.dma_start(
                            o_v[b, cb, :, h0 * W : (h0 + ROWS) * W], ob
                        )

    nc.finalize()
    return nc


def run(x: np.ndarray, weight: np.ndarray, bias: np.ndarray, **spmd_kwargs):
    x = np.ascontiguousarray(x, dtype=np.float32)
    weight = np.ascontiguousarray(weight, dtype=np.float32)
    bias = np.ascontiguousarray(bias, dtype=np.float32)

    nc = build_nc()
    in_maps = [
        {
            "x": x[c * B_LOCAL : (c + 1) * B_LOCAL],
            "weight": weight,
            "bias": bias,
        }
        for c in range(N_CORES)
    ]
    res = run_bass_kernel_spmd(
        nc, in_maps, core_ids=list(range(N_CORES)), **spmd_kwargs
    )
    out = np.concatenate([r["out"] for r in res.results], axis=0)
    return out, res


def kernel(x: np.ndarray, weight: np.ndarray, bias: np.ndarray) -> np.ndarray:
    out, _ = run(x, weight, bias)
    return out
